# revision 3
# baseline (speedup 1.0000x reference)
"""Batched dense KNN graph (dgl.knn_graph-style) on 8 Trainium2 NeuronCores.

Problem: x (8, 64, 4096) fp32, k=9. For each batch b, build the kNN graph over
the 4096 points (columns of x[b]) under squared Euclidean distance, returning
flat edge arrays (src, dst with +b*4096 offsets) and the top-k distances.

Sharding: embarrassingly parallel over batch - core b handles batch b.

Device algorithm (per core):
  negD[p, j] = 2*x_p.x_j - |x_p|^2 - |x_j|^2   (= -d[p,j], computed by PE)
  The contraction is augmented to K=66: rows 0..63 carry x, row 64/65 carry
  the -|x|^2/2 / ones pair on each side, so one matmul emits the full
  (scaled by 1/2) negative distance directly into PSUM. Scaling by 1/2 is
  rank-preserving; the final values are rescaled by -2 on output.
  Self-distance is always rank-1 of the top-9 (d(i,i)=0), so the device only
  selects top-8 over the diagonal-masked row (DVE max8 + max_index); the
  diagonal is masked to -1e9 with a GPSIMD affine_select.
"""

import numpy as np

B, C, N, KK = 8, 64, 4096, 9
P = 128          # rows per tile (partitions)
CH = 512         # matmul moving free dim (one PSUM bank of fp32)
NT = N // P      # 32 row tiles
NCH = N // CH    # 8 col chunks

_CACHE = {}


def _build():
    import concourse.bacc as bacc
    import concourse.tile as tile
    import concourse.mybir as mybir

    f32 = mybir.dt.float32
    i32 = mybir.dt.int32
    u32 = mybir.dt.uint32
    Alu = mybir.AluOpType

    nc = bacc.Bacc(
        "TRN2",
        target_bir_lowering=False,
        debug=False,
        enable_asserts=False,
        num_devices=B,
    )
    x_d = nc.dram_tensor("x", [C, N], f32, kind="ExternalInput").ap()
    off_d = nc.dram_tensor("off", [P, 1], u32, kind="ExternalInput").ap()
    src_d = nc.dram_tensor("src", [N, KK], u32, kind="ExternalOutput").ap()
    dst_d = nc.dram_tensor("dst", [N, KK], u32, kind="ExternalOutput").ap()
    topk_d = nc.dram_tensor("topk", [N, KK], f32, kind="ExternalOutput").ap()

    with tile.TileContext(nc) as tc:
        with (
            tc.tile_pool(name="consts", bufs=1) as consts,
            tc.tile_pool(name="negd_pool", bufs=3) as negd_pool,
            tc.tile_pool(name="psum_pool", bufs=8, space="PSUM") as psum_pool,
            tc.tile_pool(name="small", bufs=4) as small,
        ):
            # --- setup: load x into both operand tensors, compute -|x|^2/2 ---
            # Rows 0..63: x. Row 64 (A) / 96 (Bt): -|x|^2/2. Row 96 (A) /
            # 64 (Bt): ones. Rows 65..95 zero on A so Bt's values there are
            # irrelevant-but-finite (engine APs must start at partition
            # 0/32/64/96, hence the 64/96 placement).
            KR = 97
            A = consts.tile([KR, N], f32)
            Bt = consts.tile([KR, N], f32)
            X2 = consts.tile([C, N], f32)
            halfneg = consts.tile([C, 1], f32)
            dstall = consts.tile([P, NT], u32)  # dstall[p, r] = b*N + r*128 + p
            offb = consts.tile([P, 1], u32)

            nc.sync.dma_start(out=A[0:C, :], in_=x_d)
            nc.sync.dma_start(out=Bt[0:C, :], in_=x_d)
            nc.sync.dma_start(out=offb, in_=off_d)
            nc.vector.memset(A[64:96, :], 0.0)
            nc.vector.memset(A[96:97, :], 1.0)
            nc.vector.memset(Bt[64:96, :], 0.0)
            nc.vector.memset(Bt[64:65, :], 1.0)
            nc.vector.memset(halfneg, -0.5)
            nc.vector.tensor_mul(X2, A[0:C, :], A[0:C, :])
            for c in range(NCH):
                sl = slice(c * CH, (c + 1) * CH)
                sq_ps = psum_pool.tile([1, CH], f32, tag="mm")
                nc.tensor.matmul(
                    sq_ps, lhsT=halfneg, rhs=X2[:, sl], start=True, stop=True
                )
                nc.scalar.copy(A[64:65, sl], sq_ps)
                nc.scalar.copy(Bt[96:97, sl], sq_ps)
            nc.gpsimd.iota(dstall, pattern=[[P, NT]], base=0, channel_multiplier=1)
            nc.vector.tensor_add(dstall, dstall, offb.to_broadcast([P, NT]))

            # --- main loop over row tiles ---
            for r in range(NT):
                jd = (r * P) // CH          # col chunk containing the diagonal
                o = (r * P) % CH            # diagonal offset within that chunk
                negD = negd_pool.tile([P, N], f32, tag="negD")
                for c in range(NCH):
                    sl = slice(c * CH, (c + 1) * CH)
                    pt = psum_pool.tile([P, CH], f32, tag="mm")
                    nc.tensor.matmul(
                        pt,
                        lhsT=A[:, r * P : (r + 1) * P],
                        rhs=Bt[:, sl],
                        start=True,
                        stop=True,
                    )
                    nc.scalar.copy(negD[:, sl], pt)
                dsl = slice(jd * CH, (jd + 1) * CH)
                nc.gpsimd.affine_select(
                    out=negD[:, dsl],
                    in_=negD[:, dsl],
                    pattern=[[1, CH]],
                    compare_op=Alu.not_equal,
                    fill=-1e9,
                    base=-o,
                    channel_multiplier=-1,
                )
                v8 = small.tile([P, 8], f32, tag="v8")
                i8 = small.tile([P, 8], u32, tag="i8")
                nc.vector.max(out=v8, in_=negD)
                nc.vector.max_index(out=i8, in_max=v8, in_values=negD)

                src9 = small.tile([P, KK], u32, tag="src9")
                dst9 = small.tile([P, KK], u32, tag="dst9")
                topk9 = small.tile([P, KK], f32, tag="topk9")
                nc.vector.tensor_copy(src9[:, 0:1], dstall[:, r : r + 1])
                nc.vector.tensor_add(src9[:, 1:KK], i8, offb.to_broadcast([P, KK - 1]))
                nc.vector.memset(topk9[:, 0:1], 0.0)
                # topk values: d = -2 * negD_scaled (negD carries -d/2)
                nc.vector.tensor_scalar(topk9[:, 1:KK], v8, -2.0, None, op0=Alu.mult)
                nc.vector.tensor_copy(dst9, dstall[:, r : r + 1].to_broadcast([P, KK]))
                rsl = slice(r * P, (r + 1) * P)
                nc.sync.dma_start(out=src_d[rsl, :], in_=src9)
                nc.sync.dma_start(out=dst_d[rsl, :], in_=dst9)
                nc.sync.dma_start(out=topk_d[rsl, :], in_=topk9)

    nc.compile()
    return nc


def get_nc():
    if "nc" not in _CACHE:
        _CACHE["nc"] = _build()
    return _CACHE["nc"]


def kernel(x, k):
    import concourse.bass_utils as bass_utils

    assert int(k) == KK
    x = np.ascontiguousarray(np.asarray(x), dtype=np.float32)
    assert x.shape == (B, C, N), x.shape

    nc = get_nc()
    in_maps = [
        {
            "x": x[b],
            "off": np.full((P, 1), b * N, dtype=np.uint32),
        }
        for b in range(B)
    ]
    res = bass_utils.run_bass_kernel_spmd(nc, in_maps, core_ids=list(range(B)))
    outs = res.results
    src = np.stack([outs[b]["src"] for b in range(B)]).reshape(-1).astype(np.int32)
    dst = np.stack([outs[b]["dst"] for b in range(B)]).reshape(-1).astype(np.int32)
    topk = np.stack([outs[b]["topk"] for b in range(B)]).astype(np.float32)
    return src, dst, topk
